# revision 8
# baseline (speedup 1.0000x reference)
"""Trainium2 Bass kernel for nn_AttnBlock_12704513262242.

Math (per sample b, W=2048 "positions" with scalar q/k values):
  h   = layernorm(x) * gamma + beta
  q,k,v = h @ W* + b*
  attn  = softmax(-|q_j - k_i|, over i)
  h2[j] = sum_i attn[j,i] * v[i]
  out   = x + h2 @ Wp + bp

Sharding: feature-parallel QKV/proj (each core owns a 256-col slice of all
four weight matrices), AllToAll to redistribute q/k/v sample-major, then
pure data-parallel attention (4 samples per core), AllGather of h2, and a
feature-sliced output projection.  Host concatenates the 8 [32,256] slices.

Attention modes:
  naive  — materialize exp(-|q_j-k_i|) tiles (ACT) and reduce with PE matmuls.
  binned — softmin kernel exp(-|q-k|) factorizes as e^{-q}e^{k} (k<=q) +
           e^{q}e^{-k} (k>q).  Build cumulative tables A/C (prefix sums of
           e^k*v, e^k) and B/D (suffix sums of e^{-k}*v, e^{-k}) at G=128
           grid points via 0/1-indicator matmuls, then evaluate each query at
           its nearest grid point with a one-hot matmul whose nonzeros are
           pre-scaled by the exact e^{-+q_j}.  Quantization error ~4e-4 rel.
"""

import os
import sys

import numpy as np

for _p in ("/opt/trn_rl_repo", "/root/.axon_site/_ro/trn_rl_repo"):
    if os.path.isdir(_p) and _p not in sys.path:
        sys.path.insert(0, _p)

import concourse.bass as bass
import concourse.tile as tile
from concourse import bacc, mybir
from concourse.bass_utils import run_bass_kernel_spmd

F32 = mybir.dt.float32
F16 = mybir.dt.float16
ALU = mybir.AluOpType
ACTF = mybir.ActivationFunctionType

B = 32            # batch
W = 2048          # width (positions / features)
NCORES = 8
PCH = W // 128    # 16 partition chunks of the feature dim
FSL = W // NCORES  # 256 feature-slice per core
QKVW = 3 * FSL    # 768
SPC = B // NCORES  # 4 samples per core

G = 128           # grid bins for binned mode
LO, HI = -8.0, 8.0
DELTA = (HI - LO) / (G - 1)
HALF = DELTA / 2.0
EPS = 1e-6

MODE = os.environ.get("ATTN_MODE", "naive")
GROUPS = [list(range(NCORES))]


def _ap(tensor_handle, offset, ap):
    return bass.AP(tensor=tensor_handle, offset=offset, ap=ap)


def build(mode=None):
    mode = mode or MODE
    nc = bacc.Bacc("TRN2", target_bir_lowering=False, debug=False,
                   num_devices=NCORES)

    x_t = nc.dram_tensor("x", [B, W], F32, kind="ExternalInput")
    gamma_t = nc.dram_tensor("gamma", [W], F32, kind="ExternalInput")
    beta_t = nc.dram_tensor("beta", [W], F32, kind="ExternalInput")
    wqkv_t = nc.dram_tensor("wqkv", [W, QKVW], F32, kind="ExternalInput")
    bqkv_t = nc.dram_tensor("bqkv", [QKVW], F32, kind="ExternalInput")
    wp_t = nc.dram_tensor("wp", [W, FSL], F32, kind="ExternalInput")
    bp_t = nc.dram_tensor("bp", [FSL], F32, kind="ExternalInput")
    xs_t = nc.dram_tensor("xs", [B, FSL], F32, kind="ExternalInput")
    out_t = nc.dram_tensor("out", [B, FSL], F32, kind="ExternalOutput")

    qkv_loc = nc.dram_tensor("qkv_loc", [B, QKVW], F32)
    qkv_a2a = nc.dram_tensor("qkv_a2a", [B, QKVW], F32)
    h2_loc = nc.dram_tensor("h2_loc", [SPC, W], F32)
    h2_gat = nc.dram_tensor("h2_gat", [B, W], F32, addr_space="Shared")

    c_eye32 = nc.inline_tensor(np.eye(32, dtype=np.float32), "c_eye32")
    c_eye8 = nc.inline_tensor(np.eye(8, dtype=np.float32), "c_eye8")
    c_eye2 = nc.inline_tensor(np.eye(2, dtype=np.float32), "c_eye2")
    c_ones132 = nc.inline_tensor(np.ones((1, 32), np.float32), "c_ones132")
    gridv = np.linspace(LO, HI, G, dtype=np.float64).astype(np.float32)
    c_gcol = nc.inline_tensor(gridv.reshape(G, 1), "c_gcol")
    c_grow = nc.inline_tensor(gridv.reshape(1, G), "c_grow")

    aps = dict(
        x=x_t.ap(), gamma=gamma_t.ap(), beta=beta_t.ap(),
        wqkv=wqkv_t.ap(), bqkv=bqkv_t.ap(), wp=wp_t.ap(), bp=bp_t.ap(),
        xs=xs_t.ap(), out=out_t.ap(),
        qkv_loc=qkv_loc.ap(), qkv_a2a=qkv_a2a.ap(),
        h2_loc=h2_loc.ap(), h2_gat=h2_gat.ap(),
        eye32=c_eye32.ap(), eye8=c_eye8.ap(), eye2=c_eye2.ap(),
        ones132=c_ones132.ap(), gcol=c_gcol.ap(), grow=c_grow.ap(),
        a2a_tensor=qkv_a2a,
    )

    with tile.TileContext(nc) as tc:
        _build_tile(tc, aps, mode)

    nc.compile()
    return nc


def _build_tile(tc, aps, mode):
    nc = tc.nc

    with tc.tile_pool(name="singles", bufs=1) as singles:
        # ---- constants into SBUF ----
        eye32 = singles.tile([32, 32], F32)
        nc.sync.dma_start(eye32[:], aps["eye32"])
        eye8 = singles.tile([8, 8], F32)
        nc.sync.dma_start(eye8[:], aps["eye8"])
        eye2 = singles.tile([2, 2], F32)
        nc.sync.dma_start(eye2[:], aps["eye2"])
        ones132 = singles.tile([1, 32], F32)
        nc.sync.dma_start(ones132[:], aps["ones132"])
        gcol = singles.tile([G, 1], F32)
        nc.sync.dma_start(gcol[:], aps["gcol"])
        gbc = singles.tile([128, G], F32)
        nc.gpsimd.dma_start(gbc[:], aps["grow"].partition_broadcast(128))

        # ---- weights (issued first so DMA overlaps the rest) ----
        w32 = singles.tile([128, PCH, QKVW], F32)
        for ci in range(PCH):
            nc.sync.dma_start(w32[:, ci, :],
                              aps["wqkv"][ci * 128:(ci + 1) * 128, :])
        bq32 = singles.tile([1, QKVW], F32)
        nc.sync.dma_start(bq32[:], aps["bqkv"].partition_broadcast(1))
        wp32 = singles.tile([128, PCH, FSL], F32)
        for ci in range(PCH):
            nc.sync.dma_start(wp32[:, ci, :],
                              aps["wp"][ci * 128:(ci + 1) * 128, :])

        # residual + bp, exact fp32: xb = x_slice + bp
        xb = singles.tile([B, FSL], F32)
        bpb = singles.tile([B, FSL], F32)
        nc.gpsimd.dma_start(bpb[:], aps["bp"].partition_broadcast(B))
        xsl = singles.tile([B, FSL], F32)
        nc.sync.dma_start(xsl[:], aps["xs"])
        nc.vector.tensor_add(xb[:], xsl[:], bpb[:])

        # ---- layernorm (replicated, all 32 samples) ----
        sbx = singles.tile([B, W], F32, tag="bigio")
        nc.sync.dma_start(sbx[:], aps["x"])
        xg = sbx[:].rearrange("b (s f) -> b s f", s=4)  # 4 subgroups of 512
        stats = singles.tile([B, 4, 6], F32)
        for sg in range(4):
            nc.vector.bn_stats(stats[:, sg, :], xg[:, sg, :])
        mv = singles.tile([B, 2], F32)
        nc.vector.bn_aggr(mv[:], stats[:])
        eps_t = singles.tile([B, 1], F32)
        nc.vector.memset(eps_t[:], EPS)
        stdv = singles.tile([B, 1], F32)
        nc.scalar.activation(stdv[:], mv[:, 1:2], ACTF.Sqrt, bias=eps_t[:])
        rstd = singles.tile([B, 1], F32)
        nc.vector.reciprocal(rstd[:], stdv[:])
        h = singles.tile([B, W], F32)
        nc.vector.tensor_scalar(h[:], sbx[:], mv[:, 0:1], rstd[:],
                                op0=ALU.subtract, op1=ALU.mult)
        gb = singles.tile([B, W], F32, tag="gbb")
        nc.gpsimd.dma_start(gb[:], aps["gamma"].partition_broadcast(B))
        nc.vector.tensor_mul(h[:], h[:], gb[:])
        bb = singles.tile([B, W], F32, tag="gbb")
        nc.gpsimd.dma_start(bb[:], aps["beta"].partition_broadcast(B))
        nc.vector.tensor_add(h[:], h[:], bb[:])

        # ---- transpose h -> hT [128, PCH, 32] ----
        hT = singles.tile([128, PCH, B], F32)
        with tc.tile_pool(name="ptr", bufs=2, space="PSUM") as ptr_pool:
            for ci in range(PCH):
                ptr = ptr_pool.tile([128, B], F32)
                nc.tensor.transpose(ptr[:], h[:, ci * 128:(ci + 1) * 128],
                                    eye32[:])
                nc.vector.tensor_copy(hT[:, ci, :], ptr[:])

        # ---- qkv matmul: [32, 768] = h @ wqkv + bqkv ----
        sbq = singles.tile([B, QKVW], F32)
        with tc.tile_pool(name="pq", bufs=1, space="PSUM") as pq_pool:
            pq = pq_pool.tile([B, QKVW], F32)
            for ci in range(PCH):
                nc.tensor.matmul(pq[:, 0:512], hT[:, ci, :],
                                 w32[:, ci, 0:512],
                                 start=(ci == 0), stop=False)
                nc.tensor.matmul(pq[:, 512:QKVW], hT[:, ci, :],
                                 w32[:, ci, 512:QKVW],
                                 start=(ci == 0), stop=False)
            nc.tensor.matmul(pq[:, 0:512], ones132[:], bq32[:, 0:512],
                             start=False, stop=True)
            nc.tensor.matmul(pq[:, 512:QKVW], ones132[:], bq32[:, 512:QKVW],
                             start=False, stop=True)
            nc.vector.tensor_copy(sbq[:], pq[:])
        nc.sync.dma_start(aps["qkv_loc"], sbq[:])

        nc.gpsimd.collective_compute(
            "AllToAll", ALU.bypass, replica_groups=GROUPS,
            ins=[aps["qkv_loc"]], outs=[aps["qkv_a2a"]])

        # ---- attention (4 samples) ----
        num_t = singles.tile([SPC, W], F32)
        den_t = singles.tile([SPC, W], F32)
        shared = dict(a2a=aps["a2a_tensor"], num=num_t, den=den_t,
                      eye8=eye8, eye2=eye2, gbc=gbc, gcol=gcol)
        if mode == "binned":
            _attn_binned(tc, shared)
        else:
            _attn_naive(tc, shared)

        dinv = singles.tile([SPC, W], F32)
        nc.vector.reciprocal(dinv[:], den_t[:])
        sbh2 = singles.tile([SPC, W], F32)
        nc.vector.tensor_mul(sbh2[:], num_t[:], dinv[:])
        nc.sync.dma_start(aps["h2_loc"], sbh2[:])

        nc.gpsimd.collective_compute(
            "AllGather", ALU.bypass, replica_groups=GROUPS,
            ins=[aps["h2_loc"]], outs=[aps["h2_gat"]])

        # ---- output projection ----
        h2f = singles.tile([B, W], F32, tag="bigio")
        nc.sync.dma_start(h2f[:], aps["h2_gat"])
        h2T = singles.tile([128, PCH, B], F32)
        with tc.tile_pool(name="ptr2", bufs=2, space="PSUM") as ptr2_pool:
            for ci in range(PCH):
                ptr2 = ptr2_pool.tile([128, B], F32)
                nc.tensor.transpose(ptr2[:], h2f[:, ci * 128:(ci + 1) * 128],
                                    eye32[:])
                nc.vector.tensor_copy(h2T[:, ci, :], ptr2[:])

        sbo = singles.tile([B, FSL], F32)
        with tc.tile_pool(name="pout", bufs=1, space="PSUM") as pout_pool:
            pout = pout_pool.tile([B, FSL], F32)
            for ci in range(PCH):
                nc.tensor.matmul(pout[:], h2T[:, ci, :], wp32[:, ci, :],
                                 start=(ci == 0), stop=(ci == PCH - 1))
            nc.vector.tensor_add(sbo[:], pout[:], xb[:])
        nc.sync.dma_start(aps["out"], sbo[:])


def _load_qkv_sample(nc, kv_pool, ptp_pool, shared, s):
    """Per-sample loads from the AllToAll result: broadcast q [128, W] and
    k/v transposed into [128, 16] (feature chunk m = half*8 + coreblk)."""
    a2a = shared["a2a"]
    eye8 = shared["eye8"]
    row_k = kv_pool.tile([8, 256], F32, tag="krow")
    nc.sync.dma_start(row_k[:], _ap(a2a, s * QKVW + FSL,
                                    [[4 * QKVW, 8], [1, 256]]))
    row_v = kv_pool.tile([8, 256], F32, tag="vrow")
    nc.sync.dma_start(row_v[:], _ap(a2a, s * QKVW + 2 * FSL,
                                    [[4 * QKVW, 8], [1, 256]]))
    kTt = kv_pool.tile([128, PCH], F32, tag="kT")
    vTt = kv_pool.tile([128, PCH], F32, tag="vT")
    for half in range(2):
        ptk = ptp_pool.tile([128, 8], F32, tag="ptp")
        nc.tensor.transpose(ptk[:], row_k[:, half * 128:(half + 1) * 128],
                            eye8[:])
        nc.vector.tensor_copy(kTt[:, half * 8:(half + 1) * 8], ptk[:])
        ptv = ptp_pool.tile([128, 8], F32, tag="ptp")
        nc.tensor.transpose(ptv[:], row_v[:, half * 128:(half + 1) * 128],
                            eye8[:])
        nc.vector.tensor_copy(vTt[:, half * 8:(half + 1) * 8], ptv[:])
    return kTt, vTt


def _q_broadcast(nc, pool, shared, s, clamp):
    qb = pool.tile([128, W], F32, tag="qb")
    src = _ap(shared["a2a"], s * QKVW, [[0, 128], [4 * QKVW, 8], [1, 256]])
    nc.gpsimd.dma_start(qb[:], src)
    if clamp:
        nc.vector.tensor_scalar(qb[:], qb[:], LO, HI,
                                op0=ALU.max, op1=ALU.min)
    return qb


def _attn_binned(tc, shared):
    nc = tc.nc
    gbc = shared["gbc"]
    gcol = shared["gcol"]
    eye2 = shared["eye2"]
    with (
        tc.tile_pool(name="akv", bufs=2) as kv_pool,
        tc.tile_pool(name="aqb", bufs=2) as qb_pool,
        tc.tile_pool(name="aoh", bufs=1) as oh_pool,
        tc.tile_pool(name="amk", bufs=3) as mk_pool,
        tc.tile_pool(name="atab", bufs=2) as tab_pool,
        tc.tile_pool(name="ptp", bufs=2, space="PSUM") as ptp_pool,
        tc.tile_pool(name="ptab", bufs=2, space="PSUM") as ptab_pool,
        tc.tile_pool(name="pnd", bufs=1, space="PSUM") as pnd_pool,
    ):
        for s in range(SPC):
            qb = _q_broadcast(nc, qb_pool, shared, s, clamp=True)
            kTt, vTt = _load_qkv_sample(nc, kv_pool, ptp_pool, shared, s)

            ek = kv_pool.tile([128, PCH], F32, tag="ek")
            nc.scalar.activation(ek[:], kTt[:], ACTF.Exp)
            emk = kv_pool.tile([128, PCH], F32, tag="emk")
            nc.scalar.activation(emk[:], kTt[:], ACTF.Exp, scale=-1.0)
            u = kv_pool.tile([128, PCH, 4], F16, tag="u")
            nc.vector.tensor_mul(u[:, :, 0], ek[:], vTt[:])
            nc.vector.tensor_copy(u[:, :, 1], ek[:])
            nc.vector.tensor_mul(u[:, :, 2], emk[:], vTt[:])
            nc.vector.tensor_copy(u[:, :, 3], emk[:])

            # cumulative tables at the G grid points: psum rows = u-type
            ptab = ptab_pool.tile([4, 2 * G], F32, tag="ptab")
            for m in range(PCH):
                mk = mk_pool.tile([128, 2 * G], F16, tag="mk")
                nc.vector.tensor_scalar(mk[:, 0:G], gbc[:],
                                        kTt[:, m:m + 1], None, op0=ALU.is_ge)
                nc.vector.tensor_scalar(mk[:, G:2 * G], gbc[:],
                                        kTt[:, m:m + 1], None, op0=ALU.is_lt)
                nc.tensor.matmul(ptab[:], u[:, m, :], mk[:],
                                 start=(m == 0), stop=(m == PCH - 1))
            # rows 0,1 x cols [0,G)  = A,C (prefix with e^k);
            # rows 2,3 x cols [G,2G) = B,D (suffix with e^-k)
            sbtab = tab_pool.tile([4, 2 * G], F32, tag="sbtab")
            nc.scalar.copy(sbtab[:], ptab[:])
            sbBD = tab_pool.tile([2, G], F32, tag="sbBD")
            nc.sync.dma_start(sbBD[:], sbtab[2:4, G:2 * G])
            tabs = tab_pool.tile([G, 4], F16, tag="tabs")
            ptt = ptp_pool.tile([G, 2], F32, tag="ptp")
            nc.tensor.transpose(ptt[:], sbtab[0:2, 0:G], eye2[:])
            nc.vector.tensor_copy(tabs[:, 0:2], ptt[:])
            ptt2 = ptp_pool.tile([G, 2], F32, tag="ptp")
            nc.tensor.transpose(ptt2[:], sbBD[:], eye2[:])
            nc.vector.tensor_copy(tabs[:, 2:4], ptt2[:])

            # one-hot of nearest grid point, pre-scaled by e^{-+q}
            t1 = qb_pool.tile([128, W], F32, tag="t1", bufs=1)
            nc.vector.tensor_scalar(t1[:], qb[:], gcol[:], HALF,
                                    op0=ALU.subtract, op1=ALU.abs_max)
            oh = oh_pool.tile([128, W], F16, tag="oh")
            nc.vector.tensor_scalar(oh[:], t1[:], HALF, None, op0=ALU.is_le)
            emq = oh_pool.tile([128, W], F16, tag="emq")
            nc.scalar.activation(emq[:], qb[:], ACTF.Exp, scale=-1.0)
            epq = oh_pool.tile([128, W], F16, tag="epq")
            nc.scalar.activation(epq[:], qb[:], ACTF.Exp, scale=1.0)
            ohm = oh_pool.tile([128, W], F16, tag="ohm")
            nc.vector.tensor_mul(ohm[:], oh[:], emq[:])
            ohp = oh_pool.tile([128, W], F16, tag="ohp")
            nc.vector.tensor_mul(ohp[:], oh[:], epq[:])

            pnd = pnd_pool.tile([2, W], F32, tag="pnd")
            for n in range(4):
                sl = slice(n * 512, (n + 1) * 512)
                nc.tensor.matmul(pnd[:, sl], tabs[:, 0:2], ohm[:, sl],
                                 start=True, stop=False)
                nc.tensor.matmul(pnd[:, sl], tabs[:, 2:4], ohp[:, sl],
                                 start=False, stop=True)
            ns_s = oh_pool.tile([2, W], F32, tag="ns")
            nc.scalar.copy(ns_s[:], pnd[:])
            nc.sync.dma_start(shared["num"][s:s + 1, :], ns_s[0:1, :])
            nc.sync.dma_start(shared["den"][s:s + 1, :], ns_s[1:2, :])


def _attn_naive(tc, shared):
    nc = tc.nc
    with (
        tc.tile_pool(name="akv", bufs=2) as kv_pool,
        tc.tile_pool(name="aqb", bufs=2) as qb_pool,
        tc.tile_pool(name="aab", bufs=2) as ab_pool,
        tc.tile_pool(name="apt", bufs=3) as pt_pool,
        tc.tile_pool(name="ptp", bufs=2, space="PSUM") as ptp_pool,
        tc.tile_pool(name="pnd", bufs=1, space="PSUM") as pnd_pool,
    ):
        for s in range(SPC):
            qb = _q_broadcast(nc, qb_pool, shared, s, clamp=False)
            kTt, vTt = _load_qkv_sample(nc, kv_pool, ptp_pool, shared, s)

            nk = kv_pool.tile([128, PCH], F32, tag="nk")
            nc.vector.tensor_scalar(nk[:], kTt[:], -1.0, None, op0=ALU.mult)
            u2 = kv_pool.tile([128, PCH, 2], F16, tag="u2")
            nc.vector.tensor_copy(u2[:, :, 0], vTt[:])
            nc.vector.memset(u2[:, :, 1], 1.0)

            pnd = pnd_pool.tile([2, W], F32, tag="pnd")
            for m in range(PCH):
                ab = ab_pool.tile([128, W], F32, tag="ab")
                nc.scalar.activation(ab[:], qb[:], ACTF.Abs,
                                     bias=nk[:, m:m + 1])
                pt = pt_pool.tile([128, W], F16, tag="pt")
                nc.scalar.activation(pt[:], ab[:], ACTF.Exp, scale=-1.0)
                for n in range(4):
                    sl = slice(n * 512, (n + 1) * 512)
                    nc.tensor.matmul(pnd[:, sl], u2[:, m, :], pt[:, sl],
                                     start=(m == 0), stop=(m == PCH - 1))
            ns_s = ab_pool.tile([2, W], F32, tag="ns")
            nc.scalar.copy(ns_s[:], pnd[:])
            nc.sync.dma_start(shared["num"][s:s + 1, :], ns_s[0:1, :])
            nc.sync.dma_start(shared["den"][s:s + 1, :], ns_s[1:2, :])


_BUILT = {}


def _get_nc(mode):
    if mode not in _BUILT:
        _BUILT[mode] = build(mode)
    return _BUILT[mode]


def make_in_maps(inputs):
    x = np.ascontiguousarray(np.asarray(inputs["x"], np.float32))
    gamma = np.ascontiguousarray(np.asarray(inputs["gamma"], np.float32))
    beta = np.ascontiguousarray(np.asarray(inputs["beta"], np.float32))
    Wq = np.asarray(inputs["Wq"], np.float32)
    Wk = np.asarray(inputs["Wk"], np.float32)
    Wv = np.asarray(inputs["Wv"], np.float32)
    Wp = np.asarray(inputs["Wp"], np.float32)
    bq = np.asarray(inputs["bq"], np.float32)
    bk = np.asarray(inputs["bk"], np.float32)
    bv = np.asarray(inputs["bv"], np.float32)
    bp = np.asarray(inputs["bp"], np.float32)
    in_maps = []
    for c in range(NCORES):
        cs = slice(c * FSL, (c + 1) * FSL)
        in_maps.append({
            "x": x,
            "gamma": gamma,
            "beta": beta,
            "wqkv": np.ascontiguousarray(
                np.concatenate([Wq[:, cs], Wk[:, cs], Wv[:, cs]], axis=1)),
            "bqkv": np.ascontiguousarray(
                np.concatenate([bq[cs], bk[cs], bv[cs]])),
            "wp": np.ascontiguousarray(Wp[:, cs]),
            "bp": np.ascontiguousarray(bp[cs]),
            "xs": np.ascontiguousarray(x[:, cs]),
        })
    return in_maps


def kernel(**inputs):
    nc = _get_nc(MODE)
    in_maps = make_in_maps(inputs)
    res = run_bass_kernel_spmd(nc, in_maps, core_ids=list(range(NCORES)))
    out = np.concatenate([res.results[c]["out"] for c in range(NCORES)],
                         axis=1)
    return np.ascontiguousarray(out.astype(np.float32))


# revision 9
# speedup vs baseline: 212.1780x; 212.1780x over previous
"""Trainium2 Bass kernel for nn_AttnBlock_12704513262242.

Math (per sample b, W=2048 "positions" with scalar q/k values):
  h   = layernorm(x) * gamma + beta
  q,k,v = h @ W* + b*
  attn  = softmax(-|q_j - k_i|, over i)
  h2[j] = sum_i attn[j,i] * v[i]
  out   = x + h2 @ Wp + bp

Sharding: feature-parallel QKV/proj (each core owns a 256-col slice of all
four weight matrices), AllToAll to redistribute q/k/v sample-major, then
pure data-parallel attention (4 samples per core), AllGather of h2, and a
feature-sliced output projection.  Host concatenates the 8 [32,256] slices.

Attention modes:
  naive  — materialize exp(-|q_j-k_i|) tiles (ACT) and reduce with PE matmuls.
  binned — softmin kernel exp(-|q-k|) factorizes as e^{-q}e^{k} (k<=q) +
           e^{q}e^{-k} (k>q).  Build cumulative tables A/C (prefix sums of
           e^k*v, e^k) and B/D (suffix sums of e^{-k}*v, e^{-k}) at G=128
           grid points via 0/1-indicator matmuls, then evaluate each query at
           its nearest grid point with a one-hot matmul whose nonzeros are
           pre-scaled by the exact e^{-+q_j}.  Quantization error ~4e-4 rel.
"""

import os
import sys

import numpy as np

for _p in ("/opt/trn_rl_repo", "/root/.axon_site/_ro/trn_rl_repo"):
    if os.path.isdir(_p) and _p not in sys.path:
        sys.path.insert(0, _p)

import concourse.bass as bass
import concourse.tile as tile
from concourse import bacc, mybir
from concourse.bass_utils import run_bass_kernel_spmd

F32 = mybir.dt.float32
F16 = mybir.dt.float16
ALU = mybir.AluOpType
ACTF = mybir.ActivationFunctionType

B = 32            # batch
W = 2048          # width (positions / features)
NCORES = 8
PCH = W // 128    # 16 partition chunks of the feature dim
FSL = W // NCORES  # 256 feature-slice per core
QKVW = 3 * FSL    # 768
SPC = B // NCORES  # 4 samples per core

G = 128           # grid bins for binned mode
LO, HI = -8.0, 8.0
DELTA = (HI - LO) / (G - 1)
HALF = DELTA / 2.0
EPS = 1e-6

MODE = os.environ.get("ATTN_MODE", "naive")
GROUPS = [list(range(NCORES))]


def _ap(tensor_handle, offset, ap):
    return bass.AP(tensor=tensor_handle, offset=offset, ap=ap)


def build(mode=None, reps=1):
    mode = mode or MODE
    nc = bacc.Bacc("TRN2", target_bir_lowering=False, debug=False,
                   num_devices=NCORES)

    x_t = nc.dram_tensor("x", [B, W], F32, kind="ExternalInput")
    gamma_t = nc.dram_tensor("gamma", [W], F32, kind="ExternalInput")
    beta_t = nc.dram_tensor("beta", [W], F32, kind="ExternalInput")
    wqkv_t = nc.dram_tensor("wqkv", [W, QKVW], F32, kind="ExternalInput")
    bqkv_t = nc.dram_tensor("bqkv", [QKVW], F32, kind="ExternalInput")
    wp_t = nc.dram_tensor("wp", [W, FSL], F32, kind="ExternalInput")
    bp_t = nc.dram_tensor("bp", [FSL], F32, kind="ExternalInput")
    xs_t = nc.dram_tensor("xs", [B, FSL], F32, kind="ExternalInput")
    out_t = nc.dram_tensor("out", [B, FSL], F32, kind="ExternalOutput")

    qkv_loc = nc.dram_tensor("qkv_loc", [B, QKVW], F32)
    qkv_a2a = nc.dram_tensor("qkv_a2a", [B, QKVW], F32)
    h2_loc = nc.dram_tensor("h2_loc", [SPC, W], F32)
    h2_gat = nc.dram_tensor("h2_gat", [B, W], F32, addr_space="Shared")

    c_eye32 = nc.inline_tensor(np.eye(32, dtype=np.float32), "c_eye32")
    c_eye8 = nc.inline_tensor(np.eye(8, dtype=np.float32), "c_eye8")
    c_eye2 = nc.inline_tensor(np.eye(2, dtype=np.float32), "c_eye2")
    c_ones132 = nc.inline_tensor(np.ones((1, 32), np.float32), "c_ones132")
    gridv = np.linspace(LO, HI, G, dtype=np.float64).astype(np.float32)
    c_gcol = nc.inline_tensor(gridv.reshape(G, 1), "c_gcol")
    c_grow = nc.inline_tensor(gridv.reshape(1, G), "c_grow")

    aps = dict(
        x=x_t.ap(), gamma=gamma_t.ap(), beta=beta_t.ap(),
        wqkv=wqkv_t.ap(), bqkv=bqkv_t.ap(), wp=wp_t.ap(), bp=bp_t.ap(),
        xs=xs_t.ap(), out=out_t.ap(),
        qkv_loc=qkv_loc.ap(), qkv_a2a=qkv_a2a.ap(),
        h2_loc=h2_loc.ap(), h2_gat=h2_gat.ap(),
        eye32=c_eye32.ap(), eye8=c_eye8.ap(), eye2=c_eye2.ap(),
        ones132=c_ones132.ap(), gcol=c_gcol.ap(), grow=c_grow.ap(),
        a2a_tensor=qkv_a2a,
    )

    with tile.TileContext(nc) as tc:
        for _rep in range(reps):
            _build_tile(tc, aps, mode)

    nc.compile()
    return nc


def _build_tile(tc, aps, mode):
    nc = tc.nc

    with tc.tile_pool(name="singles", bufs=1) as singles:
        # ---- constants into SBUF ----
        eye32 = singles.tile([32, 32], F32)
        nc.sync.dma_start(eye32[:], aps["eye32"])
        eye8 = singles.tile([8, 8], F32)
        nc.sync.dma_start(eye8[:], aps["eye8"])
        eye2 = singles.tile([2, 2], F32)
        nc.sync.dma_start(eye2[:], aps["eye2"])
        ones132 = singles.tile([1, 32], F32)
        nc.sync.dma_start(ones132[:], aps["ones132"])
        gcol = singles.tile([G, 1], F32)
        nc.sync.dma_start(gcol[:], aps["gcol"])
        gbc = singles.tile([128, G], F32)
        nc.gpsimd.dma_start(gbc[:], aps["grow"].partition_broadcast(128))

        # ---- weights (issued first so DMA overlaps the rest) ----
        w32 = singles.tile([128, PCH, QKVW], F32)
        for ci in range(PCH):
            nc.sync.dma_start(w32[:, ci, :],
                              aps["wqkv"][ci * 128:(ci + 1) * 128, :])
        bq32 = singles.tile([1, QKVW], F32)
        nc.sync.dma_start(bq32[:], aps["bqkv"].partition_broadcast(1))
        wp32 = singles.tile([128, PCH, FSL], F32)
        for ci in range(PCH):
            nc.sync.dma_start(wp32[:, ci, :],
                              aps["wp"][ci * 128:(ci + 1) * 128, :])

        # residual + bp, exact fp32: xb = x_slice + bp
        xb = singles.tile([B, FSL], F32)
        bpb = singles.tile([B, FSL], F32)
        nc.gpsimd.dma_start(bpb[:], aps["bp"].partition_broadcast(B))
        xsl = singles.tile([B, FSL], F32)
        nc.sync.dma_start(xsl[:], aps["xs"])
        nc.vector.tensor_add(xb[:], xsl[:], bpb[:])

        # ---- layernorm (replicated, all 32 samples) ----
        sbx = singles.tile([B, W], F32, tag="bigio")
        nc.sync.dma_start(sbx[:], aps["x"])
        xg = sbx[:].rearrange("b (s f) -> b s f", s=4)  # 4 subgroups of 512
        stats = singles.tile([B, 4, 6], F32)
        for sg in range(4):
            nc.vector.bn_stats(stats[:, sg, :], xg[:, sg, :])
        mv = singles.tile([B, 2], F32)
        nc.vector.bn_aggr(mv[:], stats[:])
        eps_t = singles.tile([B, 1], F32)
        nc.vector.memset(eps_t[:], EPS)
        stdv = singles.tile([B, 1], F32)
        nc.scalar.activation(stdv[:], mv[:, 1:2], ACTF.Sqrt, bias=eps_t[:])
        rstd = singles.tile([B, 1], F32)
        nc.vector.reciprocal(rstd[:], stdv[:])
        h = singles.tile([B, W], F32)
        nc.vector.tensor_scalar(h[:], sbx[:], mv[:, 0:1], rstd[:],
                                op0=ALU.subtract, op1=ALU.mult)
        gb = singles.tile([B, W], F32, tag="gbb")
        nc.gpsimd.dma_start(gb[:], aps["gamma"].partition_broadcast(B))
        nc.vector.tensor_mul(h[:], h[:], gb[:])
        bb = singles.tile([B, W], F32, tag="gbb")
        nc.gpsimd.dma_start(bb[:], aps["beta"].partition_broadcast(B))
        nc.vector.tensor_add(h[:], h[:], bb[:])

        # ---- transpose h -> hT [128, PCH, 32] ----
        hT = singles.tile([128, PCH, B], F32)
        with tc.tile_pool(name="ptr", bufs=2, space="PSUM") as ptr_pool:
            for ci in range(PCH):
                ptr = ptr_pool.tile([128, B], F32)
                nc.tensor.transpose(ptr[:], h[:, ci * 128:(ci + 1) * 128],
                                    eye32[:])
                nc.vector.tensor_copy(hT[:, ci, :], ptr[:])

        # ---- qkv matmul: [32, 768] = h @ wqkv + bqkv ----
        sbq = singles.tile([B, QKVW], F32)
        with tc.tile_pool(name="pq", bufs=1, space="PSUM") as pq_pool:
            pq = pq_pool.tile([B, QKVW], F32)
            for ci in range(PCH):
                nc.tensor.matmul(pq[:, 0:512], hT[:, ci, :],
                                 w32[:, ci, 0:512],
                                 start=(ci == 0), stop=False)
                nc.tensor.matmul(pq[:, 512:QKVW], hT[:, ci, :],
                                 w32[:, ci, 512:QKVW],
                                 start=(ci == 0), stop=False)
            nc.tensor.matmul(pq[:, 0:512], ones132[:], bq32[:, 0:512],
                             start=False, stop=True)
            nc.tensor.matmul(pq[:, 512:QKVW], ones132[:], bq32[:, 512:QKVW],
                             start=False, stop=True)
            nc.vector.tensor_copy(sbq[:], pq[:])
        nc.sync.dma_start(aps["qkv_loc"], sbq[:])

        nc.gpsimd.collective_compute(
            "AllToAll", ALU.bypass, replica_groups=GROUPS,
            ins=[aps["qkv_loc"]], outs=[aps["qkv_a2a"]])

        # ---- attention (4 samples) ----
        num_t = singles.tile([SPC, W], F32)
        den_t = singles.tile([SPC, W], F32)
        shared = dict(a2a=aps["a2a_tensor"], num=num_t, den=den_t,
                      eye8=eye8, eye2=eye2, gbc=gbc, gcol=gcol)
        if mode == "binned":
            _attn_binned(tc, shared)
        else:
            _attn_naive(tc, shared)

        dinv = singles.tile([SPC, W], F32)
        nc.vector.reciprocal(dinv[:], den_t[:])
        sbh2 = singles.tile([SPC, W], F32)
        nc.vector.tensor_mul(sbh2[:], num_t[:], dinv[:])
        nc.sync.dma_start(aps["h2_loc"], sbh2[:])

        nc.gpsimd.collective_compute(
            "AllGather", ALU.bypass, replica_groups=GROUPS,
            ins=[aps["h2_loc"]], outs=[aps["h2_gat"]])

        # ---- output projection ----
        h2f = singles.tile([B, W], F32, tag="bigio")
        nc.sync.dma_start(h2f[:], aps["h2_gat"])
        h2T = singles.tile([128, PCH, B], F32)
        with tc.tile_pool(name="ptr2", bufs=2, space="PSUM") as ptr2_pool:
            for ci in range(PCH):
                ptr2 = ptr2_pool.tile([128, B], F32)
                nc.tensor.transpose(ptr2[:], h2f[:, ci * 128:(ci + 1) * 128],
                                    eye32[:])
                nc.vector.tensor_copy(h2T[:, ci, :], ptr2[:])

        sbo = singles.tile([B, FSL], F32)
        with tc.tile_pool(name="pout", bufs=1, space="PSUM") as pout_pool:
            pout = pout_pool.tile([B, FSL], F32)
            for ci in range(PCH):
                nc.tensor.matmul(pout[:], h2T[:, ci, :], wp32[:, ci, :],
                                 start=(ci == 0), stop=(ci == PCH - 1))
            nc.vector.tensor_add(sbo[:], pout[:], xb[:])
        nc.sync.dma_start(aps["out"], sbo[:])


def _load_qkv_sample(nc, kv_pool, ptp_pool, shared, s):
    """Per-sample loads from the AllToAll result: broadcast q [128, W] and
    k/v transposed into [128, 16] (feature chunk m = half*8 + coreblk)."""
    a2a = shared["a2a"]
    eye8 = shared["eye8"]
    row_k = kv_pool.tile([8, 256], F32, tag="krow")
    nc.sync.dma_start(row_k[:], _ap(a2a, s * QKVW + FSL,
                                    [[4 * QKVW, 8], [1, 256]]))
    row_v = kv_pool.tile([8, 256], F32, tag="vrow")
    nc.sync.dma_start(row_v[:], _ap(a2a, s * QKVW + 2 * FSL,
                                    [[4 * QKVW, 8], [1, 256]]))
    kTt = kv_pool.tile([128, PCH], F32, tag="kT")
    vTt = kv_pool.tile([128, PCH], F32, tag="vT")
    for half in range(2):
        ptk = ptp_pool.tile([128, 8], F32, tag="ptp")
        nc.tensor.transpose(ptk[:], row_k[:, half * 128:(half + 1) * 128],
                            eye8[:])
        nc.vector.tensor_copy(kTt[:, half * 8:(half + 1) * 8], ptk[:])
        ptv = ptp_pool.tile([128, 8], F32, tag="ptp")
        nc.tensor.transpose(ptv[:], row_v[:, half * 128:(half + 1) * 128],
                            eye8[:])
        nc.vector.tensor_copy(vTt[:, half * 8:(half + 1) * 8], ptv[:])
    return kTt, vTt


def _q_broadcast(nc, pool, shared, s, clamp):
    qb = pool.tile([128, W], F32, tag="qb")
    src = _ap(shared["a2a"], s * QKVW, [[0, 128], [4 * QKVW, 8], [1, 256]])
    nc.gpsimd.dma_start(qb[:], src)
    if clamp:
        nc.vector.tensor_scalar(qb[:], qb[:], LO, HI,
                                op0=ALU.max, op1=ALU.min)
    return qb


def _attn_binned(tc, shared):
    nc = tc.nc
    gbc = shared["gbc"]
    gcol = shared["gcol"]
    eye2 = shared["eye2"]
    with (
        tc.tile_pool(name="akv", bufs=2) as kv_pool,
        tc.tile_pool(name="aqb", bufs=2) as qb_pool,
        tc.tile_pool(name="aoh", bufs=1) as oh_pool,
        tc.tile_pool(name="amk", bufs=3) as mk_pool,
        tc.tile_pool(name="atab", bufs=2) as tab_pool,
        tc.tile_pool(name="ptp", bufs=2, space="PSUM") as ptp_pool,
        tc.tile_pool(name="ptab", bufs=2, space="PSUM") as ptab_pool,
        tc.tile_pool(name="pnd", bufs=1, space="PSUM") as pnd_pool,
    ):
        for s in range(SPC):
            qb = _q_broadcast(nc, qb_pool, shared, s, clamp=True)
            kTt, vTt = _load_qkv_sample(nc, kv_pool, ptp_pool, shared, s)

            ek = kv_pool.tile([128, PCH], F32, tag="ek")
            nc.scalar.activation(ek[:], kTt[:], ACTF.Exp)
            emk = kv_pool.tile([128, PCH], F32, tag="emk")
            nc.scalar.activation(emk[:], kTt[:], ACTF.Exp, scale=-1.0)
            u = kv_pool.tile([128, PCH, 4], F16, tag="u")
            nc.vector.tensor_mul(u[:, :, 0], ek[:], vTt[:])
            nc.vector.tensor_copy(u[:, :, 1], ek[:])
            nc.vector.tensor_mul(u[:, :, 2], emk[:], vTt[:])
            nc.vector.tensor_copy(u[:, :, 3], emk[:])

            # cumulative tables at the G grid points: psum rows = u-type
            ptab = ptab_pool.tile([4, 2 * G], F32, tag="ptab")
            for m in range(PCH):
                mk = mk_pool.tile([128, 2 * G], F16, tag="mk")
                nc.vector.tensor_scalar(mk[:, 0:G], gbc[:],
                                        kTt[:, m:m + 1], None, op0=ALU.is_ge)
                nc.vector.tensor_scalar(mk[:, G:2 * G], gbc[:],
                                        kTt[:, m:m + 1], None, op0=ALU.is_lt)
                nc.tensor.matmul(ptab[:], u[:, m, :], mk[:],
                                 start=(m == 0), stop=(m == PCH - 1))
            # rows 0,1 x cols [0,G)  = A,C (prefix with e^k);
            # rows 2,3 x cols [G,2G) = B,D (suffix with e^-k)
            sbtab = tab_pool.tile([4, 2 * G], F32, tag="sbtab")
            nc.scalar.copy(sbtab[:], ptab[:])
            sbBD = tab_pool.tile([2, G], F32, tag="sbBD")
            nc.sync.dma_start(sbBD[:], sbtab[2:4, G:2 * G])
            tabs = tab_pool.tile([G, 4], F16, tag="tabs")
            ptt = ptp_pool.tile([G, 2], F32, tag="ptp")
            nc.tensor.transpose(ptt[:], sbtab[0:2, 0:G], eye2[:])
            nc.vector.tensor_copy(tabs[:, 0:2], ptt[:])
            ptt2 = ptp_pool.tile([G, 2], F32, tag="ptp")
            nc.tensor.transpose(ptt2[:], sbBD[:], eye2[:])
            nc.vector.tensor_copy(tabs[:, 2:4], ptt2[:])

            # one-hot of nearest grid point, pre-scaled by e^{-+q}
            t1 = qb_pool.tile([128, W], F32, tag="t1", bufs=1)
            nc.vector.tensor_scalar(t1[:], qb[:], gcol[:], HALF,
                                    op0=ALU.subtract, op1=ALU.abs_max)
            oh = oh_pool.tile([128, W], F16, tag="oh")
            nc.vector.tensor_scalar(oh[:], t1[:], HALF, None, op0=ALU.is_le)
            emq = oh_pool.tile([128, W], F16, tag="emq")
            nc.scalar.activation(emq[:], qb[:], ACTF.Exp, scale=-1.0)
            epq = oh_pool.tile([128, W], F16, tag="epq")
            nc.scalar.activation(epq[:], qb[:], ACTF.Exp, scale=1.0)
            ohm = oh_pool.tile([128, W], F16, tag="ohm")
            nc.vector.tensor_mul(ohm[:], oh[:], emq[:])
            ohp = oh_pool.tile([128, W], F16, tag="ohp")
            nc.vector.tensor_mul(ohp[:], oh[:], epq[:])

            pnd = pnd_pool.tile([2, W], F32, tag="pnd")
            for n in range(4):
                sl = slice(n * 512, (n + 1) * 512)
                nc.tensor.matmul(pnd[:, sl], tabs[:, 0:2], ohm[:, sl],
                                 start=True, stop=False)
                nc.tensor.matmul(pnd[:, sl], tabs[:, 2:4], ohp[:, sl],
                                 start=False, stop=True)
            ns_s = oh_pool.tile([2, W], F32, tag="ns")
            nc.scalar.copy(ns_s[:], pnd[:])
            nc.sync.dma_start(shared["num"][s:s + 1, :], ns_s[0:1, :])
            nc.sync.dma_start(shared["den"][s:s + 1, :], ns_s[1:2, :])


def _attn_naive(tc, shared):
    nc = tc.nc
    with (
        tc.tile_pool(name="akv", bufs=2) as kv_pool,
        tc.tile_pool(name="aqb", bufs=2) as qb_pool,
        tc.tile_pool(name="aab", bufs=2) as ab_pool,
        tc.tile_pool(name="apt", bufs=3) as pt_pool,
        tc.tile_pool(name="ptp", bufs=2, space="PSUM") as ptp_pool,
        tc.tile_pool(name="pnd", bufs=1, space="PSUM") as pnd_pool,
    ):
        for s in range(SPC):
            qb = _q_broadcast(nc, qb_pool, shared, s, clamp=False)
            kTt, vTt = _load_qkv_sample(nc, kv_pool, ptp_pool, shared, s)

            nk = kv_pool.tile([128, PCH], F32, tag="nk")
            nc.vector.tensor_scalar(nk[:], kTt[:], -1.0, None, op0=ALU.mult)
            u2 = kv_pool.tile([128, PCH, 2], F16, tag="u2")
            nc.vector.tensor_copy(u2[:, :, 0], vTt[:])
            nc.vector.memset(u2[:, :, 1], 1.0)

            pnd = pnd_pool.tile([2, W], F32, tag="pnd")
            for m in range(PCH):
                ab = ab_pool.tile([128, W], F32, tag="ab")
                nc.scalar.activation(ab[:], qb[:], ACTF.Abs,
                                     bias=nk[:, m:m + 1])
                pt = pt_pool.tile([128, W], F16, tag="pt")
                nc.scalar.activation(pt[:], ab[:], ACTF.Exp, scale=-1.0)
                for n in range(4):
                    sl = slice(n * 512, (n + 1) * 512)
                    nc.tensor.matmul(pnd[:, sl], u2[:, m, :], pt[:, sl],
                                     start=(m == 0), stop=(m == PCH - 1))
            ns_s = ab_pool.tile([2, W], F32, tag="ns")
            nc.scalar.copy(ns_s[:], pnd[:])
            nc.sync.dma_start(shared["num"][s:s + 1, :], ns_s[0:1, :])
            nc.sync.dma_start(shared["den"][s:s + 1, :], ns_s[1:2, :])


_BUILT = {}


def _get_nc(mode):
    if mode not in _BUILT:
        _BUILT[mode] = build(mode)
    return _BUILT[mode]


def make_in_maps(inputs):
    x = np.ascontiguousarray(np.asarray(inputs["x"], np.float32))
    gamma = np.ascontiguousarray(np.asarray(inputs["gamma"], np.float32))
    beta = np.ascontiguousarray(np.asarray(inputs["beta"], np.float32))
    Wq = np.asarray(inputs["Wq"], np.float32)
    Wk = np.asarray(inputs["Wk"], np.float32)
    Wv = np.asarray(inputs["Wv"], np.float32)
    Wp = np.asarray(inputs["Wp"], np.float32)
    bq = np.asarray(inputs["bq"], np.float32)
    bk = np.asarray(inputs["bk"], np.float32)
    bv = np.asarray(inputs["bv"], np.float32)
    bp = np.asarray(inputs["bp"], np.float32)
    in_maps = []
    for c in range(NCORES):
        cs = slice(c * FSL, (c + 1) * FSL)
        in_maps.append({
            "x": x,
            "gamma": gamma,
            "beta": beta,
            "wqkv": np.ascontiguousarray(
                np.concatenate([Wq[:, cs], Wk[:, cs], Wv[:, cs]], axis=1)),
            "bqkv": np.ascontiguousarray(
                np.concatenate([bq[cs], bk[cs], bv[cs]])),
            "wp": np.ascontiguousarray(Wp[:, cs]),
            "bp": np.ascontiguousarray(bp[cs]),
            "xs": np.ascontiguousarray(x[:, cs]),
        })
    return in_maps


def kernel(**inputs):
    nc = _get_nc(MODE)
    in_maps = make_in_maps(inputs)
    res = run_bass_kernel_spmd(nc, in_maps, core_ids=list(range(NCORES)))
    out = np.concatenate([res.results[c]["out"] for c in range(NCORES)],
                         axis=1)
    return np.ascontiguousarray(out.astype(np.float32))


# revision 10
# speedup vs baseline: 1497.5027x; 7.0578x over previous
"""Trainium2 Bass kernel for nn_AttnBlock_12704513262242.

Math (per sample b, W=2048 "positions" with scalar q/k values):
  h   = layernorm(x) * gamma + beta
  q,k,v = h @ W* + b*
  attn  = softmax(-|q_j - k_i|, over i)
  h2[j] = sum_i attn[j,i] * v[i]
  out   = x + h2 @ Wp + bp

Sharding: feature-parallel QKV/proj (each core owns a 256-col slice of all
four weight matrices), AllToAll to redistribute q/k/v sample-major, then
pure data-parallel attention (4 samples per core), AllGather of h2, and a
feature-sliced output projection.  Host concatenates the 8 [32,256] slices.

Attention modes:
  naive  — materialize exp(-|q_j-k_i|) tiles (ACT) and reduce with PE matmuls.
  binned — softmin kernel exp(-|q-k|) factorizes as e^{-q}e^{k} (k<=q) +
           e^{q}e^{-k} (k>q).  Build cumulative tables A/C (prefix sums of
           e^k*v, e^k) and B/D (suffix sums of e^{-k}*v, e^{-k}) at G=128
           grid points via 0/1-indicator matmuls, then evaluate each query at
           its nearest grid point with a one-hot matmul whose nonzeros are
           pre-scaled by the exact e^{-+q_j}.  Quantization error ~4e-4 rel.
"""

import os
import sys

import numpy as np

for _p in ("/opt/trn_rl_repo", "/root/.axon_site/_ro/trn_rl_repo"):
    if os.path.isdir(_p) and _p not in sys.path:
        sys.path.insert(0, _p)

import concourse.bass as bass
import concourse.tile as tile
from concourse import bacc, mybir
from concourse.bass_utils import run_bass_kernel_spmd

F32 = mybir.dt.float32
F16 = mybir.dt.float16
ALU = mybir.AluOpType
ACTF = mybir.ActivationFunctionType

B = 32            # batch
W = 2048          # width (positions / features)
NCORES = 8
PCH = W // 128    # 16 partition chunks of the feature dim
FSL = W // NCORES  # 256 feature-slice per core
QKVW = 3 * FSL    # 768
SPC = B // NCORES  # 4 samples per core

G = 128           # grid bins for binned mode
LO, HI = -8.0, 8.0
DELTA = (HI - LO) / (G - 1)
HALF = DELTA / 2.0
EPS = 1e-6

MODE = os.environ.get("ATTN_MODE", "naive")
GROUPS = [list(range(NCORES))]


def _ap(tensor_handle, offset, ap):
    return bass.AP(tensor=tensor_handle, offset=offset, ap=ap)


def build(mode=None, reps=1):
    mode = mode or MODE
    nc = bacc.Bacc("TRN2", target_bir_lowering=False, debug=False,
                   num_devices=NCORES)

    x_t = nc.dram_tensor("x", [B, W], F32, kind="ExternalInput")
    gamma_t = nc.dram_tensor("gamma", [W], F32, kind="ExternalInput")
    beta_t = nc.dram_tensor("beta", [W], F32, kind="ExternalInput")
    wqkv_t = nc.dram_tensor("wqkv", [W, QKVW], F32, kind="ExternalInput")
    bqkv_t = nc.dram_tensor("bqkv", [QKVW], F32, kind="ExternalInput")
    wp_t = nc.dram_tensor("wp", [W, FSL], F32, kind="ExternalInput")
    bp_t = nc.dram_tensor("bp", [FSL], F32, kind="ExternalInput")
    xs_t = nc.dram_tensor("xs", [B, FSL], F32, kind="ExternalInput")
    out_t = nc.dram_tensor("out", [B, FSL], F32, kind="ExternalOutput")

    qkv_loc = nc.dram_tensor("qkv_loc", [B, QKVW], F32)
    qkv_a2a = nc.dram_tensor("qkv_a2a", [B, QKVW], F32)
    h2_loc = nc.dram_tensor("h2_loc", [SPC, W], F32)
    h2_gat = nc.dram_tensor("h2_gat", [B, W], F32, addr_space="Shared")

    c_eye32 = nc.inline_tensor(np.eye(32, dtype=np.float32), "c_eye32")
    c_eye8 = nc.inline_tensor(np.eye(8, dtype=np.float32), "c_eye8")
    c_eye2 = nc.inline_tensor(np.eye(2, dtype=np.float32), "c_eye2")
    c_ones132 = nc.inline_tensor(np.ones((1, 32), np.float32), "c_ones132")
    gridv = np.linspace(LO, HI, G, dtype=np.float64).astype(np.float32)
    c_gcol = nc.inline_tensor(gridv.reshape(G, 1), "c_gcol")
    c_gcoln = nc.inline_tensor(-gridv.reshape(G, 1), "c_gcoln")
    c_grow = nc.inline_tensor(gridv.reshape(1, G), "c_grow")

    aps = dict(
        x=x_t.ap(), gamma=gamma_t.ap(), beta=beta_t.ap(),
        wqkv=wqkv_t.ap(), bqkv=bqkv_t.ap(), wp=wp_t.ap(), bp=bp_t.ap(),
        xs=xs_t.ap(), out=out_t.ap(),
        qkv_loc=qkv_loc.ap(), qkv_a2a=qkv_a2a.ap(),
        h2_loc=h2_loc.ap(), h2_gat=h2_gat.ap(),
        eye32=c_eye32.ap(), eye8=c_eye8.ap(), eye2=c_eye2.ap(),
        ones132=c_ones132.ap(), gcol=c_gcol.ap(), gcoln=c_gcoln.ap(),
        grow=c_grow.ap(),
        a2a_tensor=qkv_a2a,
    )

    with tile.TileContext(nc) as tc:
        for _rep in range(reps):
            _build_tile(tc, aps, mode)

    nc.compile()
    return nc


def _build_tile(tc, aps, mode):
    nc = tc.nc

    with tc.tile_pool(name="singles", bufs=1) as singles:
        # ---- constants into SBUF ----
        eye32 = singles.tile([32, 32], F32)
        nc.sync.dma_start(eye32[:], aps["eye32"])
        eye8 = singles.tile([8, 8], F32)
        nc.sync.dma_start(eye8[:], aps["eye8"])
        eye2 = singles.tile([2, 2], F32)
        nc.sync.dma_start(eye2[:], aps["eye2"])
        ones132 = singles.tile([1, 32], F32)
        nc.sync.dma_start(ones132[:], aps["ones132"])
        gcol = singles.tile([G, 1], F32)
        nc.sync.dma_start(gcol[:], aps["gcol"])
        gcoln = singles.tile([G, 1], F32)
        nc.sync.dma_start(gcoln[:], aps["gcoln"])
        gbc = singles.tile([128, G], F32)
        nc.gpsimd.dma_start(gbc[:], aps["grow"].partition_broadcast(128))

        # ---- weights (issued first so DMA overlaps the rest) ----
        w32 = singles.tile([128, PCH, QKVW], F32)
        for ci in range(PCH):
            nc.sync.dma_start(w32[:, ci, :],
                              aps["wqkv"][ci * 128:(ci + 1) * 128, :])
        bq32 = singles.tile([1, QKVW], F32)
        nc.sync.dma_start(bq32[:], aps["bqkv"].partition_broadcast(1))
        wp32 = singles.tile([128, PCH, FSL], F32)
        for ci in range(PCH):
            nc.sync.dma_start(wp32[:, ci, :],
                              aps["wp"][ci * 128:(ci + 1) * 128, :])

        # residual + bp, exact fp32: xb = x_slice + bp
        xb = singles.tile([B, FSL], F32)
        bpb = singles.tile([B, FSL], F32)
        nc.gpsimd.dma_start(bpb[:], aps["bp"].partition_broadcast(B))
        xsl = singles.tile([B, FSL], F32)
        nc.sync.dma_start(xsl[:], aps["xs"])
        nc.vector.tensor_add(xb[:], xsl[:], bpb[:])

        # ---- layernorm (replicated, all 32 samples) ----
        sbx = singles.tile([B, W], F32, tag="bigio")
        nc.sync.dma_start(sbx[:], aps["x"])
        xg = sbx[:].rearrange("b (s f) -> b s f", s=4)  # 4 subgroups of 512
        stats = singles.tile([B, 4, 6], F32)
        for sg in range(4):
            nc.vector.bn_stats(stats[:, sg, :], xg[:, sg, :])
        mv = singles.tile([B, 2], F32)
        nc.vector.bn_aggr(mv[:], stats[:])
        eps_t = singles.tile([B, 1], F32)
        nc.vector.memset(eps_t[:], EPS)
        stdv = singles.tile([B, 1], F32)
        nc.scalar.activation(stdv[:], mv[:, 1:2], ACTF.Sqrt, bias=eps_t[:])
        rstd = singles.tile([B, 1], F32)
        nc.vector.reciprocal(rstd[:], stdv[:])
        h = singles.tile([B, W], F32)
        nc.vector.tensor_scalar(h[:], sbx[:], mv[:, 0:1], rstd[:],
                                op0=ALU.subtract, op1=ALU.mult)
        gb = singles.tile([B, W], F32, tag="gbb")
        nc.gpsimd.dma_start(gb[:], aps["gamma"].partition_broadcast(B))
        nc.vector.tensor_mul(h[:], h[:], gb[:])
        bb = singles.tile([B, W], F32, tag="gbb")
        nc.gpsimd.dma_start(bb[:], aps["beta"].partition_broadcast(B))
        nc.vector.tensor_add(h[:], h[:], bb[:])

        # ---- transpose h -> hT [128, PCH, 32] ----
        hT = singles.tile([128, PCH, B], F32)
        with tc.tile_pool(name="ptr", bufs=2, space="PSUM") as ptr_pool:
            for ci in range(PCH):
                ptr = ptr_pool.tile([128, B], F32)
                nc.tensor.transpose(ptr[:], h[:, ci * 128:(ci + 1) * 128],
                                    eye32[:])
                nc.vector.tensor_copy(hT[:, ci, :], ptr[:])

        # ---- qkv matmul: [32, 768] = h @ wqkv + bqkv ----
        sbq = singles.tile([B, QKVW], F32)
        with tc.tile_pool(name="pq", bufs=1, space="PSUM") as pq_pool:
            pq = pq_pool.tile([B, QKVW], F32)
            for ci in range(PCH):
                nc.tensor.matmul(pq[:, 0:512], hT[:, ci, :],
                                 w32[:, ci, 0:512],
                                 start=(ci == 0), stop=False)
                nc.tensor.matmul(pq[:, 512:QKVW], hT[:, ci, :],
                                 w32[:, ci, 512:QKVW],
                                 start=(ci == 0), stop=False)
            nc.tensor.matmul(pq[:, 0:512], ones132[:], bq32[:, 0:512],
                             start=False, stop=True)
            nc.tensor.matmul(pq[:, 512:QKVW], ones132[:], bq32[:, 512:QKVW],
                             start=False, stop=True)
            nc.vector.tensor_copy(sbq[:], pq[:])
        nc.sync.dma_start(aps["qkv_loc"], sbq[:])

        nc.gpsimd.collective_compute(
            "AllToAll", ALU.bypass, replica_groups=GROUPS,
            ins=[aps["qkv_loc"]], outs=[aps["qkv_a2a"]])

        # ---- attention (4 samples) ----
        num_t = singles.tile([SPC, W], F32)
        den_t = singles.tile([SPC, W], F32)
        shared = dict(a2a=aps["a2a_tensor"], num=num_t, den=den_t,
                      eye8=eye8, eye2=eye2, gbc=gbc, gcol=gcol, gcoln=gcoln)
        if mode == "binned":
            _attn_binned(tc, shared)
        else:
            _attn_naive(tc, shared)

        dinv = singles.tile([SPC, W], F32)
        nc.vector.reciprocal(dinv[:], den_t[:])
        sbh2 = singles.tile([SPC, W], F32)
        nc.vector.tensor_mul(sbh2[:], num_t[:], dinv[:])
        nc.sync.dma_start(aps["h2_loc"], sbh2[:])

        nc.gpsimd.collective_compute(
            "AllGather", ALU.bypass, replica_groups=GROUPS,
            ins=[aps["h2_loc"]], outs=[aps["h2_gat"]])

        # ---- output projection ----
        h2f = singles.tile([B, W], F32, tag="bigio")
        nc.sync.dma_start(h2f[:], aps["h2_gat"])
        h2T = singles.tile([128, PCH, B], F32)
        with tc.tile_pool(name="ptr2", bufs=2, space="PSUM") as ptr2_pool:
            for ci in range(PCH):
                ptr2 = ptr2_pool.tile([128, B], F32)
                nc.tensor.transpose(ptr2[:], h2f[:, ci * 128:(ci + 1) * 128],
                                    eye32[:])
                nc.vector.tensor_copy(h2T[:, ci, :], ptr2[:])

        sbo = singles.tile([B, FSL], F32)
        with tc.tile_pool(name="pout", bufs=1, space="PSUM") as pout_pool:
            pout = pout_pool.tile([B, FSL], F32)
            for ci in range(PCH):
                nc.tensor.matmul(pout[:], h2T[:, ci, :], wp32[:, ci, :],
                                 start=(ci == 0), stop=(ci == PCH - 1))
            nc.vector.tensor_add(sbo[:], pout[:], xb[:])
        nc.sync.dma_start(aps["out"], sbo[:])


def _load_qkv_sample(nc, kv_pool, ptp_pool, shared, s):
    """Per-sample loads from the AllToAll result: broadcast q [128, W] and
    k/v transposed into [128, 16] (feature chunk m = half*8 + coreblk)."""
    a2a = shared["a2a"]
    eye8 = shared["eye8"]
    row_k = kv_pool.tile([8, 256], F32, tag="krow")
    nc.sync.dma_start(row_k[:], _ap(a2a, s * QKVW + FSL,
                                    [[4 * QKVW, 8], [1, 256]]))
    row_v = kv_pool.tile([8, 256], F32, tag="vrow")
    nc.sync.dma_start(row_v[:], _ap(a2a, s * QKVW + 2 * FSL,
                                    [[4 * QKVW, 8], [1, 256]]))
    kTt = kv_pool.tile([128, PCH], F32, tag="kT")
    vTt = kv_pool.tile([128, PCH], F32, tag="vT")
    for half in range(2):
        ptk = ptp_pool.tile([128, 8], F32, tag="ptp")
        nc.tensor.transpose(ptk[:], row_k[:, half * 128:(half + 1) * 128],
                            eye8[:])
        nc.vector.tensor_copy(kTt[:, half * 8:(half + 1) * 8], ptk[:])
        ptv = ptp_pool.tile([128, 8], F32, tag="ptp")
        nc.tensor.transpose(ptv[:], row_v[:, half * 128:(half + 1) * 128],
                            eye8[:])
        nc.vector.tensor_copy(vTt[:, half * 8:(half + 1) * 8], ptv[:])
    return kTt, vTt


def _q_broadcast(nc, pool, shared, s, clamp):
    qb = pool.tile([128, W], F32, tag="qb")
    src = _ap(shared["a2a"], s * QKVW, [[0, 128], [4 * QKVW, 8], [1, 256]])
    nc.gpsimd.dma_start(qb[:], src)
    if clamp:
        nc.vector.tensor_scalar(qb[:], qb[:], LO, HI,
                                op0=ALU.max, op1=ALU.min)
    return qb


def _attn_binned(tc, shared):
    nc = tc.nc
    gbc = shared["gbc"]
    gcoln = shared["gcoln"]
    eye2 = shared["eye2"]
    with (
        tc.tile_pool(name="akv", bufs=2) as kv_pool,
        tc.tile_pool(name="aqb", bufs=2) as qb_pool,
        tc.tile_pool(name="aoh", bufs=1) as oh_pool,
        tc.tile_pool(name="amk", bufs=3) as mk_pool,
        tc.tile_pool(name="atab", bufs=2) as tab_pool,
        tc.tile_pool(name="ptp", bufs=2, space="PSUM") as ptp_pool,
        tc.tile_pool(name="ptab", bufs=2, space="PSUM") as ptab_pool,
        tc.tile_pool(name="pnd", bufs=1, space="PSUM") as pnd_pool,
    ):
        for s in range(SPC):
            qb = _q_broadcast(nc, qb_pool, shared, s, clamp=True)
            kTt, vTt = _load_qkv_sample(nc, kv_pool, ptp_pool, shared, s)

            ek = kv_pool.tile([128, PCH], F32, tag="ek")
            nc.scalar.activation(ek[:], kTt[:], ACTF.Exp)
            emk = kv_pool.tile([128, PCH], F32, tag="emk")
            nc.scalar.activation(emk[:], kTt[:], ACTF.Exp, scale=-1.0)
            u = kv_pool.tile([128, PCH, 4], F16, tag="u")
            nc.vector.tensor_mul(u[:, :, 0], ek[:], vTt[:])
            nc.vector.tensor_copy(u[:, :, 1], ek[:])
            nc.vector.tensor_mul(u[:, :, 2], emk[:], vTt[:])
            nc.vector.tensor_copy(u[:, :, 3], emk[:])

            # cumulative tables at the G grid points: psum rows = u-type
            ptab = ptab_pool.tile([4, 2 * G], F32, tag="ptab")
            for m in range(PCH):
                mk = mk_pool.tile([128, 2 * G], F16, tag="mk")
                nc.vector.tensor_scalar(mk[:, 0:G], gbc[:],
                                        kTt[:, m:m + 1], None, op0=ALU.is_ge)
                nc.vector.tensor_scalar(mk[:, G:2 * G], gbc[:],
                                        kTt[:, m:m + 1], None, op0=ALU.is_lt)
                nc.tensor.matmul(ptab[:], u[:, m, :], mk[:],
                                 start=(m == 0), stop=(m == PCH - 1))
            # rows 0,1 x cols [0,G)  = A,C (prefix with e^k);
            # rows 2,3 x cols [G,2G) = B,D (suffix with e^-k)
            sbtab = tab_pool.tile([4, 2 * G], F32, tag="sbtab")
            nc.scalar.copy(sbtab[:], ptab[:])
            sbBD = tab_pool.tile([2, G], F32, tag="sbBD")
            nc.sync.dma_start(sbBD[:], sbtab[2:4, G:2 * G])
            tabs = tab_pool.tile([G, 4], F16, tag="tabs")
            ptt = ptp_pool.tile([G, 2], F32, tag="ptp")
            nc.tensor.transpose(ptt[:], sbtab[0:2, 0:G], eye2[:])
            nc.vector.tensor_copy(tabs[:, 0:2], ptt[:])
            ptt2 = ptp_pool.tile([G, 2], F32, tag="ptp")
            nc.tensor.transpose(ptt2[:], sbBD[:], eye2[:])
            nc.vector.tensor_copy(tabs[:, 2:4], ptt2[:])

            # one-hot of nearest grid point, pre-scaled by e^{-+q}
            t1 = qb_pool.tile([128, W], F32, tag="t1", bufs=1)
            nc.scalar.activation(t1[:], qb[:], ACTF.Abs, bias=gcoln[:])
            oh = oh_pool.tile([128, W], F16, tag="oh")
            nc.vector.tensor_scalar(oh[:], t1[:], HALF, None, op0=ALU.is_le)
            emq = oh_pool.tile([128, W], F16, tag="emq")
            nc.scalar.activation(emq[:], qb[:], ACTF.Exp, scale=-1.0)
            epq = oh_pool.tile([128, W], F16, tag="epq")
            nc.scalar.activation(epq[:], qb[:], ACTF.Exp, scale=1.0)
            ohm = oh_pool.tile([128, W], F16, tag="ohm")
            nc.vector.tensor_mul(ohm[:], oh[:], emq[:])
            ohp = oh_pool.tile([128, W], F16, tag="ohp")
            nc.vector.tensor_mul(ohp[:], oh[:], epq[:])

            pnd = pnd_pool.tile([2, W], F32, tag="pnd")
            for n in range(4):
                sl = slice(n * 512, (n + 1) * 512)
                nc.tensor.matmul(pnd[:, sl], tabs[:, 0:2], ohm[:, sl],
                                 start=True, stop=False)
                nc.tensor.matmul(pnd[:, sl], tabs[:, 2:4], ohp[:, sl],
                                 start=False, stop=True)
            ns_s = oh_pool.tile([2, W], F32, tag="ns")
            nc.scalar.copy(ns_s[:], pnd[:])
            nc.sync.dma_start(shared["num"][s:s + 1, :], ns_s[0:1, :])
            nc.sync.dma_start(shared["den"][s:s + 1, :], ns_s[1:2, :])


def _attn_naive(tc, shared):
    nc = tc.nc
    with (
        tc.tile_pool(name="akv", bufs=2) as kv_pool,
        tc.tile_pool(name="aqb", bufs=2) as qb_pool,
        tc.tile_pool(name="aab", bufs=2) as ab_pool,
        tc.tile_pool(name="apt", bufs=3) as pt_pool,
        tc.tile_pool(name="ptp", bufs=2, space="PSUM") as ptp_pool,
        tc.tile_pool(name="pnd", bufs=1, space="PSUM") as pnd_pool,
    ):
        for s in range(SPC):
            qb = _q_broadcast(nc, qb_pool, shared, s, clamp=False)
            kTt, vTt = _load_qkv_sample(nc, kv_pool, ptp_pool, shared, s)

            nk = kv_pool.tile([128, PCH], F32, tag="nk")
            nc.vector.tensor_scalar(nk[:], kTt[:], -1.0, None, op0=ALU.mult)
            u2 = kv_pool.tile([128, PCH, 2], F16, tag="u2")
            nc.vector.tensor_copy(u2[:, :, 0], vTt[:])
            nc.vector.memset(u2[:, :, 1], 1.0)

            pnd = pnd_pool.tile([2, W], F32, tag="pnd")
            for m in range(PCH):
                ab = ab_pool.tile([128, W], F32, tag="ab")
                nc.scalar.activation(ab[:], qb[:], ACTF.Abs,
                                     bias=nk[:, m:m + 1])
                pt = pt_pool.tile([128, W], F16, tag="pt")
                nc.scalar.activation(pt[:], ab[:], ACTF.Exp, scale=-1.0)
                for n in range(4):
                    sl = slice(n * 512, (n + 1) * 512)
                    nc.tensor.matmul(pnd[:, sl], u2[:, m, :], pt[:, sl],
                                     start=(m == 0), stop=(m == PCH - 1))
            ns_s = ab_pool.tile([2, W], F32, tag="ns")
            nc.scalar.copy(ns_s[:], pnd[:])
            nc.sync.dma_start(shared["num"][s:s + 1, :], ns_s[0:1, :])
            nc.sync.dma_start(shared["den"][s:s + 1, :], ns_s[1:2, :])


_BUILT = {}


def _get_nc(mode):
    if mode not in _BUILT:
        _BUILT[mode] = build(mode)
    return _BUILT[mode]


def make_in_maps(inputs):
    x = np.ascontiguousarray(np.asarray(inputs["x"], np.float32))
    gamma = np.ascontiguousarray(np.asarray(inputs["gamma"], np.float32))
    beta = np.ascontiguousarray(np.asarray(inputs["beta"], np.float32))
    Wq = np.asarray(inputs["Wq"], np.float32)
    Wk = np.asarray(inputs["Wk"], np.float32)
    Wv = np.asarray(inputs["Wv"], np.float32)
    Wp = np.asarray(inputs["Wp"], np.float32)
    bq = np.asarray(inputs["bq"], np.float32)
    bk = np.asarray(inputs["bk"], np.float32)
    bv = np.asarray(inputs["bv"], np.float32)
    bp = np.asarray(inputs["bp"], np.float32)
    in_maps = []
    for c in range(NCORES):
        cs = slice(c * FSL, (c + 1) * FSL)
        in_maps.append({
            "x": x,
            "gamma": gamma,
            "beta": beta,
            "wqkv": np.ascontiguousarray(
                np.concatenate([Wq[:, cs], Wk[:, cs], Wv[:, cs]], axis=1)),
            "bqkv": np.ascontiguousarray(
                np.concatenate([bq[cs], bk[cs], bv[cs]])),
            "wp": np.ascontiguousarray(Wp[:, cs]),
            "bp": np.ascontiguousarray(bp[cs]),
            "xs": np.ascontiguousarray(x[:, cs]),
        })
    return in_maps


def kernel(**inputs):
    nc = _get_nc(MODE)
    in_maps = make_in_maps(inputs)
    res = run_bass_kernel_spmd(nc, in_maps, core_ids=list(range(NCORES)))
    out = np.concatenate([res.results[c]["out"] for c in range(NCORES)],
                         axis=1)
    return np.ascontiguousarray(out.astype(np.float32))
